# revision 29
# baseline (speedup 1.0000x reference)
"""Causal single-head attention on 8 trn2 NeuronCores, data-parallel over batch.

Per core (one batch element, C=2048 ctx, E=1024 emb, D=1024 query_dim):

Key algebraic restructure vs the straightforward q/k formulation: since
scores = (x Qw + qb)(x Kw + kb)^T, precompute M8 = Qw Kw^T (a [E,E] matmul,
half the cost of a [C,E]x[E,D] projection) and compute

    scores[c,j] = x_c M8 x_j^T + x_j . (Kw qb)   (+ c-only terms that cancel
                                                  in softmax and are dropped)

so only ONE [C,E]x[E,E] projection (u = x @ M8) is needed on the scores path
instead of two (q and k).  Scores are computed TRANSPOSED (j on partitions):
  scoresT chunk = xt_chunk^T @ uT, so exp() writes E^T directly in the layout
the output matmul needs as its stationary operand - no PE transposes of E and
no psum->sbuf E^T copies.  The per-key bias x_j.(Kw qb) is a per-partition
bias on the exp activation.  The softmax denominator (a sum over keys =
partitions) is an extra width-1 accumulation matmul (E^T chunk @ ones) per
key-chunk, sharing the stationary with the output matmuls.

The score path (uT projection and scoresT matmuls) runs in fp8e4m3 with
perf_mode=DoubleRow (two 128-deep k-tiles per instruction, ~1.4x PE
throughput).  M8 entries are prescaled by 64 into fp8 normal range (folded
back in the uT psum->sbuf copy).  Measured rel err vs a float64 reference:
1.81e-02 (vs 3.8e-04 all-fp16) -- softmax-weight perturbations of ~1%;
scores errors scale by 1/sqrt(D) so fp8 is affordable here, while V/E-path
fp8 would hit the output directly (~3.7%) and stays fp16.

Phases:
  P_X: cast x to fp16 (DVE), PE-transpose -> xt resident (also the scoresT
       stationary; fp8 paired copy for DoubleRow), interleaved with P_V so
       the PE never idles at startup.
  P_V: v = x @ Vw (Vw streamed in column halves so work starts after 2MB).
       Vb folded into the v tiles (weights sum to 1).
  W:   load Qw,Kw, PE-transpose them, M8 = QwT^T@KwT (psum-accum over d),
       bvec = Kw qb (width-1 matmuls), sbias = x bvec (width-1 matmuls,
       pre-scaled by 1/sqrt(D) into the exp bias).
  U:   uT = (x @ M8)^T resident (fp8 DoubleRow), psum->sbuf copies
       alternating ACT/DVE so neither engine gates the PE.
  A:   per 512-col query group g: scoresT chunks (fp8 DoubleRow, tapered at
       the diagonal; the causal mask is a DVE multiply of the exp'd diagonal
       corner by a 0/1 triangle, no PE mask matmul), exp with
       per-partition bias -> ET sbuf tiles; then per
       128-row query block: out = sum_j ET_j^T @ v_j (fp16) + denominator
       via ET_j^T @ ones, epilogue split ACT/DVE (scale by 1/den), DMA out.
       Last group runs blocks descending and the final block splits by
       column half so the tail epilogue+DMA overlaps matmuls.
"""

import os
import sys

for _p in ("/opt/trn_rl_repo", "/root/.axon_site/_ro/trn_rl_repo"):
    if os.path.isdir(_p) and _p not in sys.path:
        sys.path.insert(0, _p)

from contextlib import ExitStack

import numpy as np

import concourse.bass as bass
import concourse.tile as tile
from concourse import bacc, mybir
from concourse.masks import make_identity

F32 = mybir.dt.float32
F8 = mybir.dt.float8e4
AF = mybir.ActivationFunctionType
DR = mybir.MatmulPerfMode.DoubleRow
DTYPES = {"fp16": mybir.dt.float16, "bf16": mybir.dt.bfloat16}

P = 128


def build(C=2048, E=1024, D=1024, n_cores=8, loop=1, dt="fp16", marks=None,
          knobs=None):
    DT = DTYPES[dt]
    CC = 512            # c-chunk width for P_X / uT / P_V passes
    GW = 512            # attention query-group width
    NG = C // GW        # 4
    EC = E // P         # 8 contraction chunks (e over E)
    FC = E // P         # 8 f-chunks (cols of M8)
    RB = C // P         # 16 query row blocks
    NJ = 512
    ND = D // NJ        # 2
    NCC = C // CC
    scale = float(D) ** -0.5

    knobs = knobs or {}
    phases = knobs.get("phases", "xvwua")
    f8u = bool(knobs.get("f8u", 0))     # uT projection in fp8 DoubleRow
    f8s = bool(knobs.get("f8s", 0))     # scoresT matmuls in fp8 DoubleRow
    M8S = 64.0                          # fp8 prescale for M8 entries
    nc = bacc.Bacc("TRN2", target_bir_lowering=False, debug=False,
                   num_devices=n_cores)
    x_d = nc.dram_tensor("x", [C, E], F32, kind="ExternalInput").ap()
    qw_d = nc.dram_tensor("Qw", [E, D], F32, kind="ExternalInput").ap()
    qb_d = nc.dram_tensor("Qb", [D], F32, kind="ExternalInput").ap()
    kw_d = nc.dram_tensor("Kw", [E, D], F32, kind="ExternalInput").ap()
    kb_d = nc.dram_tensor("Kb", [D], F32, kind="ExternalInput").ap()
    vw_d = nc.dram_tensor("Vw", [E, D], F32, kind="ExternalInput").ap()
    vb_d = nc.dram_tensor("Vb", [D], F32, kind="ExternalInput").ap()
    out_d = nc.dram_tensor("out", [C, D], F32, kind="ExternalOutput").ap()

    def mark(label):
        if marks is not None:
            marks.append((label, nc.next_id()))

    with tile.TileContext(nc) as tc, ExitStack() as ctx:
        if loop > 1:
            ctx.enter_context(tc.For_i(0, loop, 1))
        const_pool = ctx.enter_context(tc.tile_pool(name="const", bufs=1))
        xt_pool = ctx.enter_context(tc.tile_pool(name="xt", bufs=1))
        v_pool = ctx.enter_context(tc.tile_pool(name="v", bufs=1))
        m8_pool = ctx.enter_context(tc.tile_pool(name="m8", bufs=1))

        # ---- constants
        ident_f = const_pool.tile([P, P], F32, name="ident_f")
        make_identity(nc, ident_f)
        ident_h = const_pool.tile([P, P], DT, name="ident_h")
        nc.vector.tensor_copy(ident_h[:], ident_f[:])
        # 0/1 causal mask for the scoresT diagonal corner (keep col >= row):
        # applied as a DVE elementwise multiply after exp.
        tri_f = const_pool.tile([P, P], F32, name="tri_f")
        nc.gpsimd.memset(tri_f[:], 1.0)
        nc.gpsimd.affine_select(
            out=tri_f[:], in_=tri_f[:], compare_op=mybir.AluOpType.is_ge,
            fill=0.0, base=0, pattern=[[1, P]], channel_multiplier=-1)
        tri_h = const_pool.tile([P, P], DT, name="tri_h")
        nc.vector.tensor_copy(tri_h[:], tri_f[:])
        ones_col = const_pool.tile([P, 1], DT, name="ones_col")
        nc.vector.memset(ones_col[:], 1.0)
        vb_f = const_pool.tile([1, D], F32, name="vb_f")
        vb_bc = const_pool.tile([P, D], F32, name="vb_bc")
        qb_h = const_pool.tile([P, EC], DT, name="qb_h")
        bvec_h = const_pool.tile([P, FC], DT, name="bvec_h")
        sbias_sb = const_pool.tile([P, RB], F32, name="sbias_sb")

        # ---- resident tensors
        xt = [xt_pool.tile([P, C], DT, name=f"xt{e}") for e in range(EC)]
        v_sb = [v_pool.tile([P, D], DT, name=f"v{i}") for i in range(RB)]
        m8 = [m8_pool.tile([P, D], DT, name=f"m8_{e}") for e in range(EC)]
        # fp8 DoubleRow operand copies: pair-of-k-tiles layout [P, 2*N] where
        # column s*N+n holds k-tile (2*i+s), viewed as [P, 2, N] at matmul
        # time via rearrange.
        if f8u or f8s:
            xt8 = [xt_pool.tile([P, 2 * C], F8, name=f"xt8_{e2}")
                   for e2 in range(EC // 2)]
            xt8v = [t.rearrange("p (s c) -> p s c", s=2) for t in xt8]
        if f8u:
            m88 = [m8_pool.tile([P, 2 * D], F8, name=f"m88_{e2}")
                   for e2 in range(EC // 2)]
            m88v = [t.rearrange("p (s c) -> p s c", s=2) for t in m88]

        # =================== P_X + P_V + W (interleaved) ===================
        # Single DMA queue; queue order == emission order:
        #   x cc0 | vb | Vw h0 | x cc1..3 | Qw h0 | Kw h0 | Vw h1 | Qw h1 |
        #   Kw h1 | qb
        # PE order interleaves P_X transposes, P_V halves, Qw/Kw transposes,
        # then M8 so the PE always has work that its DMA has already fed.
        mark("px_pv")
        with tc.tile_pool(name="px_in", bufs=3) as xin_pool, \
             tc.tile_pool(name="px_h", bufs=6) as xh_pool, \
             tc.tile_pool(name="px_ps", bufs=4, space="PSUM") as pxps_pool, \
             tc.tile_pool(name="pv_ps", bufs=4, space="PSUM") as pvps_pool, \
             tc.tile_pool(name="w_st", bufs=3) as wstp, \
             tc.tile_pool(name="w_h", bufs=1) as whp, \
             tc.tile_pool(name="w_t", bufs=1) as wtp:

            qwt = [wtp.tile([P, E], DT, name=f"qwt{d}") for d in range(EC)]
            kwt = [wtp.tile([P, E], DT, name=f"kwt{d}") for d in range(EC)]

            def px_cc(cc):
                # row-major transpose order: each transpose is gated only on
                # its own row's DMA+cast, so the PE starts ~1.5us after the
                # first 512KB rather than waiting for the whole 2MB chunk.
                xhs = []
                for cs in range(CC // P):
                    xrow = xin_pool.tile([P, E], F32, tag="xrow")
                    nc.sync.dma_start(
                        xrow[:],
                        x_d[cc * CC + cs * P: cc * CC + (cs + 1) * P, :])
                    xh = xh_pool.tile([P, E], DT, tag="xh")
                    nc.vector.tensor_copy(xh[:], xrow[:])
                    xhs.append(xh)
                for eh in range(2):
                    psts = [pxps_pool.tile([P, CC], DT, tag="pst",
                                           name=f"pst{el}")
                            for el in range(EC // 2)]
                    for cs in range(CC // P):
                        for el in range(EC // 2):
                            e = eh * (EC // 2) + el
                            nc.tensor.transpose(
                                psts[el][:, cs * P:(cs + 1) * P],
                                xhs[cs][:, e * P:(e + 1) * P], ident_h[:])
                    for el in range(EC // 2):
                        e = eh * (EC // 2) + el
                        nc.scalar.copy(
                            xt[e][:, cc * CC:(cc + 1) * CC], psts[el][:])
                        if f8u or f8s:
                            nc.vector.tensor_copy(
                                xt8[e // 2][:, (e % 2) * C + cc * CC:
                                            (e % 2) * C + (cc + 1) * CC],
                                psts[el][:])

            def load_half(w_dram, pname, h):
                H = D // 2
                tiles = []
                for e in range(EC):
                    wst = wstp.tile([P, H], F32, tag="wst",
                                    name=f"{pname}st{h}_{e}")
                    nc.sync.dma_start(
                        wst[:], w_dram[e * P:(e + 1) * P, h * H:(h + 1) * H])
                    wt_ = whp.tile([P, H], DT, tag=f"{pname}h{e}",
                                   name=f"{pname}{h}_{e}")
                    nc.vector.tensor_copy(wt_[:], wst[:])
                    tiles.append(wt_)
                return tiles

            def pv_ct(ct, h, vw_half):
                # v_sb holds v + vb: out = (E^T (v+vb))/den == out + vb since
                # softmax weights sum to 1 -- no per-block vb add in the
                # epilogue.
                ps = pvps_pool.tile([P, NJ], F32, tag="ps")
                for e in range(EC):
                    nc.tensor.matmul(
                        ps[:], xt[e][:, ct * P:(ct + 1) * P], vw_half[e][:],
                        start=(e == 0), stop=(e == EC - 1))
                nc.vector.tensor_add(
                    v_sb[ct][:, h * NJ:(h + 1) * NJ], ps[:],
                    vb_bc[:, h * NJ:(h + 1) * NJ])

            def tpose_half(src, dst, h):
                # dst[d] for d in [h*EC/2, h*EC/2 + EC/2): PE-transpose the
                # column half h of the natural-layout fp16 tiles.  e-outer
                # order so each transpose is gated only on its own source
                # chunk's DMA+cast.
                for eq in range(E // CC):
                    psts = [pxps_pool.tile([P, CC], DT, tag="pst",
                                           name=f"tps{dl}")
                            for dl in range(EC // 2)]
                    for es in range(CC // P):
                        e = eq * (CC // P) + es
                        for dl in range(EC // 2):
                            nc.tensor.transpose(
                                psts[dl][:, es * P:(es + 1) * P],
                                src[e][:, dl * P:(dl + 1) * P], ident_h[:])
                    for dl in range(EC // 2):
                        nc.scalar.copy(
                            dst[h * (EC // 2) + dl][:, eq * CC:(eq + 1) * CC],
                            psts[dl][:])

            px_cc(0)
            if "v" in phases:
                nc.sync.dma_start(vb_f[:], vb_d[None, :])
                nc.gpsimd.partition_broadcast(vb_bc[:], vb_f[0:1, :])
                vw0 = load_half(vw_d, "vw", 0)
                pv_ready = True
            else:
                pv_ready = False
            for cc in range(1, NCC):
                if pv_ready:
                    for ct in range((cc - 1) * (CC // P), cc * (CC // P)):
                        pv_ct(ct, 0, vw0)
                px_cc(cc)
            if pv_ready:
                for ct in range((NCC - 1) * (CC // P), NCC * (CC // P)):
                    pv_ct(ct, 0, vw0)

            mark("w")
            if "w" in phases:
                qw0 = load_half(qw_d, "qw", 0)
                kw0 = load_half(kw_d, "kw", 0)
                tpose_half(qw0, qwt, 0)
                tpose_half(kw0, kwt, 0)
            if pv_ready:
                vw1 = load_half(vw_d, "vw", 1)
                for ct in range(RB):
                    pv_ct(ct, 1, vw1)
            if "w" in phases:
                qw1 = load_half(qw_d, "qw", 1)
                kw1 = load_half(kw_d, "kw", 1)
                tpose_half(qw1, qwt, 1)
                tpose_half(kw1, kwt, 1)

                # M8[e,f] = sum_d Qw[e,d] Kw[f,d]
                for e in range(EC):
                    for fc in range(E // CC):
                        ps = pvps_pool.tile([P, CC], F32, tag="ps")
                        for d in range(EC):
                            nc.tensor.matmul(
                                ps[:, :], qwt[d][:, e * P:(e + 1) * P],
                                kwt[d][:, fc * CC:(fc + 1) * CC],
                                start=(d == 0), stop=(d == EC - 1))
                        if f8u:
                            # prescaled by M8S so small M8 entries stay in
                            # e4m3 normal range; folded back out in the uT
                            # psum->sbuf copy.
                            nc.scalar.activation(
                                m88[e // 2][:, (e % 2) * D + fc * CC:
                                            (e % 2) * D + (fc + 1) * CC],
                                ps[:], AF.Identity, scale=M8S)
                        else:
                            nc.scalar.copy(
                                m8[e][:, fc * CC:(fc + 1) * CC], ps[:])

                # qb -> [P, EC] (partition-major chunks) via contiguous
                # DMA + small PE transpose (strided [D] DMA is slow).
                qb_row = const_pool.tile([EC, P], F32, name="qb_row")
                nc.sync.dma_start(
                    qb_row[:], qb_d.rearrange("(c p) -> c p", p=P))
                psq = pvps_pool.tile([P, NJ], F32, tag="ps", name="ps_qb")
                nc.tensor.transpose(psq[:, 0:EC], qb_row[:],
                                    ident_f[:EC, :EC])
                nc.scalar.copy(qb_h[:], psq[:, 0:EC])

                # bvec[f] = sum_d Kw[f,d] qb[d]  (width-1 matmuls)
                psv = pvps_pool.tile([P, NJ], F32, tag="ps", name="ps_bv")
                for fi in range(FC):
                    for d in range(EC):
                        nc.tensor.matmul(
                            psv[:, fi:fi + 1],
                            kwt[d][:, fi * P:(fi + 1) * P],
                            qb_h[:, d:d + 1],
                            start=(d == 0), stop=(d == EC - 1))
                nc.scalar.copy(bvec_h[:], psv[:, 0:FC])

                # sbias[j] = x_j . bvec, pre-scaled by 1/sqrt(D)
                pss = pvps_pool.tile([P, NJ], F32, tag="ps", name="ps_sb")
                for ct in range(RB):
                    for f in range(FC):
                        nc.tensor.matmul(
                            pss[:, ct:ct + 1],
                            xt[f][:, ct * P:(ct + 1) * P],
                            bvec_h[:, f:f + 1],
                            start=(f == 0), stop=(f == FC - 1))
                nc.scalar.activation(sbias_sb[:], pss[:, 0:RB], AF.Identity,
                                     scale=scale)
            else:
                nc.vector.memset(sbias_sb[:], 0.0)
                for tl in m8:
                    nc.gpsimd.memset(tl[:], 0.01)
                if not pv_ready:
                    nc.gpsimd.memset(vb_bc[:], 0.0)

        # =================== U: uT = (x @ M8)^T ===================
        mark("u")
        ut_pool = ctx.enter_context(tc.tile_pool(name="ut", bufs=1))
        if not f8s:
            ut = [ut_pool.tile([P, C], DT, name=f"ut{f}") for f in range(FC)]
        else:
            ut8 = [ut_pool.tile([P, 2 * C], F8, name=f"ut8_{f2}")
                   for f2 in range(FC // 2)]
            ut8v = [t.rearrange("p (s c) -> p s c", s=2) for t in ut8]
        if "u" in phases:
            uscale = (1.0 / M8S) if f8u else 1.0
            with tc.tile_pool(name="u_ps", bufs=4, space="PSUM") as ups:
                for cc in range(NCC):
                    for f in range(FC):
                        ps = ups.tile([P, CC], F32, tag="ps")
                        if f8u:
                            for e2 in range(EC // 2):
                                nc.tensor.matmul(
                                    ps[:],
                                    m88v[e2][:, :, f * P:(f + 1) * P],
                                    xt8v[e2][:, :, cc * CC:(cc + 1) * CC],
                                    perf_mode=DR,
                                    start=(e2 == 0), stop=(e2 == EC // 2 - 1))
                        else:
                            for e in range(EC):
                                nc.tensor.matmul(
                                    ps[:], m8[e][:, f * P:(f + 1) * P],
                                    xt[e][:, cc * CC:(cc + 1) * CC],
                                    start=(e == 0), stop=(e == EC - 1))
                        if f8s:
                            dst = ut8[f // 2][:, (f % 2) * C + cc * CC:
                                              (f % 2) * C + (cc + 1) * CC]
                        else:
                            dst = ut[f][:, cc * CC:(cc + 1) * CC]
                        # alternate copy engine so neither ACT nor DVE gates
                        # the projection matmuls
                        if f % 2 == 0:
                            nc.scalar.activation(dst, ps[:], AF.Identity,
                                                 scale=uscale)
                        else:
                            nc.vector.tensor_scalar_mul(dst, ps[:], uscale)
        else:
            for tl in (ut8 if f8s else ut):
                nc.gpsimd.memset(tl[:], 0.01)

        # =================== A: attention ===================
        mark("attn")
        if "a" in phases:
            with tc.tile_pool(name="et", bufs=knobs.get("et", 1)) as et_pool, \
                 tc.tile_pool(name="os", bufs=knobs.get("os", 2)) as os_pool, \
                 tc.tile_pool(name="r", bufs=knobs.get("r", 2)) as r_pool, \
                 tc.tile_pool(name="a_s", bufs=knobs.get("s", 2),
                              space="PSUM") as s_pool, \
                 tc.tile_pool(name="a_o", bufs=knobs.get("o", 2),
                              space="PSUM") as o_pool, \
                 tc.tile_pool(name="a_d", bufs=knobs.get("dn", 2),
                              space="PSUM") as d_pool:

                for g in range(NG):
                    gc0 = g * GW
                    njj = (g + 1) * (GW // P)       # j-chunks this group
                    ets = [et_pool.tile([P, GW], DT, tag=f"et{jj}",
                                        name=f"et{jj}")
                           for jj in range(njj)]
                    # scoresT chunks + exp
                    for jj in range(njj):
                        diag = jj * P >= gc0
                        c0 = max(gc0, jj * P)
                        w = (g + 1) * GW - c0
                        loc0 = c0 - gc0
                        ps_s = s_pool.tile([P, GW], F32, tag="ps_s")
                        if f8s:
                            for f2 in range(FC // 2):
                                nc.tensor.matmul(
                                    ps_s[:, :w],
                                    xt8v[f2][:, :, jj * P:(jj + 1) * P],
                                    ut8v[f2][:, :, c0:c0 + w],
                                    perf_mode=DR,
                                    start=(f2 == 0),
                                    stop=(f2 == FC // 2 - 1))
                        else:
                            for f in range(FC):
                                nc.tensor.matmul(
                                    ps_s[:, :w],
                                    xt[f][:, jj * P:(jj + 1) * P],
                                    ut[f][:, c0:c0 + w],
                                    start=(f == 0),
                                    stop=(f == FC - 1))
                        nc.scalar.activation(
                            ets[jj][:, loc0:loc0 + w], ps_s[:, :w], AF.Exp,
                            scale=scale, bias=sbias_sb[:, jj:jj + 1])
                        if diag:
                            # causal mask: zero the strictly-lower triangle
                            # (key j > query c) of the diagonal corner with a
                            # low-latency DVE multiply instead of an extra PE
                            # mask matmul.
                            corner = ets[jj][:, loc0:loc0 + P]
                            nc.vector.tensor_mul(corner, corner, tri_h[:])
                    # out + denominator per 128-row query block.  Last group
                    # runs blocks in descending order so the biggest block's
                    # epilogue+DMA overlaps the others' matmuls (shorter tail).
                    blocks = list(range(g * (GW // P), (g + 1) * (GW // P)))
                    if g == NG - 1:
                        blocks = blocks[::-1]
                    for bi, i in enumerate(blocks):
                        last = (g == NG - 1 and bi == len(blocks) - 1)
                        lcb = i * P - gc0
                        ps_o = [o_pool.tile([P, NJ], F32, tag=f"ps_o{dh}",
                                            name=f"ps_o{dh}")
                                for dh in range(ND)]
                        ps_den = d_pool.tile([P, 1], F32, tag="ps_den")
                        rinv = r_pool.tile([P, 1], F32, tag="rinv")
                        outst = os_pool.tile([P, D], F32, tag="outst")
                        if not last:
                            for jj in range(i + 1):
                                st = ets[jj][:, lcb:lcb + P]
                                for dh in range(ND):
                                    nc.tensor.matmul(
                                        ps_o[dh][:], st,
                                        v_sb[jj][:, dh * NJ:(dh + 1) * NJ],
                                        start=(jj == 0), stop=(jj == i))
                                nc.tensor.matmul(
                                    ps_den[:], st, ones_col[:],
                                    start=(jj == 0), stop=(jj == i))
                            nc.vector.reciprocal(rinv[:], ps_den[:])
                            # epilogue halves on different engines (DVE/ACT)
                            nc.vector.tensor_scalar_mul(
                                outst[:, 0:NJ], ps_o[0][:], rinv[:])
                            nc.scalar.activation(
                                outst[:, NJ:2 * NJ], ps_o[1][:], AF.Identity,
                                scale=rinv[:])
                            nc.sync.dma_start(out_d[i * P:(i + 1) * P, :],
                                              outst[:])
                        else:
                            # very last block: split by column half so the
                            # first half's epilogue+DMA overlaps the second
                            # half's matmuls (shorter kernel tail).
                            for jj in range(i + 1):
                                st = ets[jj][:, lcb:lcb + P]
                                nc.tensor.matmul(
                                    ps_o[0][:], st, v_sb[jj][:, 0:NJ],
                                    start=(jj == 0), stop=(jj == i))
                                nc.tensor.matmul(
                                    ps_den[:], st, ones_col[:],
                                    start=(jj == 0), stop=(jj == i))
                            nc.vector.reciprocal(rinv[:], ps_den[:])
                            nc.vector.tensor_scalar_mul(
                                outst[:, 0:NJ], ps_o[0][:], rinv[:])
                            nc.sync.dma_start(
                                out_d[i * P:(i + 1) * P, 0:NJ],
                                outst[:, 0:NJ])
                            for jj in range(i + 1):
                                nc.tensor.matmul(
                                    ps_o[1][:], ets[jj][:, lcb:lcb + P],
                                    v_sb[jj][:, NJ:2 * NJ],
                                    start=(jj == 0), stop=(jj == i))
                            nc.scalar.activation(
                                outst[:, NJ:2 * NJ], ps_o[1][:], AF.Identity,
                                scale=rinv[:])
                            nc.sync.dma_start(
                                out_d[i * P:(i + 1) * P, NJ:2 * NJ],
                                outst[:, NJ:2 * NJ])
        else:
            with tc.tile_pool(name="os", bufs=1) as os_pool:
                outst = os_pool.tile([P, D], F32, tag="outst")
                nc.vector.memset(outst[:], 0.0)
                nc.sync.dma_start(out_d[0:P, :], outst[:])
        mark("end")

    nc.compile()
    return nc


_CACHE = {}


def _built(C=2048, E=1024, D=1024, n_cores=8, loop=1, dt="fp16", knobs=None):
    key = (C, E, D, n_cores, loop, dt,
           tuple(sorted((knobs or {}).items())))
    if key not in _CACHE:
        _CACHE[key] = build(C, E, D, n_cores, loop, dt, knobs=knobs)
    return _CACHE[key]


def _executable(C=2048, E=1024, D=1024, n_cores=8, loop=1, dt="fp16",
                knobs=None):
    """Cached jitted SPMD executable for the built Bass module."""
    key = ("exec", C, E, D, n_cores, loop, dt,
           tuple(sorted((knobs or {}).items())))
    if key in _CACHE:
        return _CACHE[key]
    import jax
    from jax.sharding import Mesh, PartitionSpec
    from jax.experimental.shard_map import shard_map
    from concourse import bass2jax, mybir as _mybir

    nc = _built(C, E, D, n_cores, loop, dt, knobs=knobs)
    bass2jax.install_neuronx_cc_hook()

    partition_name = (nc.partition_id_tensor.name
                      if nc.partition_id_tensor else None)
    in_names, out_names, out_avals, zero_outs = [], [], [], []
    for alloc in nc.m.functions[0].allocations:
        if not isinstance(alloc, _mybir.MemoryLocationSet):
            continue
        name = alloc.memorylocations[0].name
        if alloc.kind == "ExternalInput":
            if name != partition_name:
                in_names.append(name)
        elif alloc.kind == "ExternalOutput":
            out_names.append(name)
            shape = tuple(alloc.tensor_shape)
            dtype = _mybir.dt.np(alloc.dtype)
            out_avals.append(jax.core.ShapedArray(shape, dtype))
            zero_outs.append(np.zeros(shape, dtype))
    n_params = len(in_names)
    all_names = in_names + out_names
    if partition_name is not None:
        all_names = all_names + [partition_name]

    def _body(*args):
        operands = list(args)
        if partition_name is not None:
            operands.append(bass2jax.partition_id_tensor())
        outs = bass2jax._bass_exec_p.bind(
            *operands,
            out_avals=tuple(out_avals),
            in_names=tuple(all_names),
            out_names=tuple(out_names),
            lowering_input_output_aliases=(),
            sim_require_finite=True,
            sim_require_nnan=True,
            nc=nc,
        )
        return tuple(outs)

    devices = jax.devices()[:n_cores]
    mesh = Mesh(np.asarray(devices), ("core",))
    n_outs = len(out_names)
    sharded = jax.jit(
        shard_map(_body, mesh=mesh,
                  in_specs=(PartitionSpec("core"),) * (n_params + n_outs),
                  out_specs=(PartitionSpec("core"),) * n_outs,
                  check_rep=False),
        donate_argnums=tuple(range(n_params, n_params + n_outs)),
        keep_unused=True,
    )
    res = dict(fn=sharded, in_names=in_names, out_names=out_names,
               out_avals=out_avals, zero_outs=zero_outs, mesh=mesh,
               n_cores=n_cores)
    _CACHE[key] = res
    return res


KNOBS = {"f8u": 1, "f8s": 1}
if os.environ.get("BASS_KNOBS"):
    import json as _json
    KNOBS = _json.loads(os.environ["BASS_KNOBS"])


def run(inputs, C=2048, E=1024, D=1024, n_cores=8, dt="fp16"):
    ex = _executable(C, E, D, n_cores, 1, dt, knobs=KNOBS or None)
    B = inputs["x"].shape[0]
    assert B == n_cores
    f = lambda a: np.ascontiguousarray(np.asarray(a, dtype=np.float32))
    shared = {k: f(inputs[k]) for k in ("Qw", "Qb", "Kw", "Kb", "Vw", "Vb")}
    x = f(inputs["x"])
    per_core = [dict(x=x[b], **shared) for b in range(B)]
    concat_in = [
        np.concatenate([per_core[c][n] for c in range(n_cores)], axis=0)
        for n in ex["in_names"]
    ]
    concat_zeros = [
        np.zeros((n_cores * z.shape[0], *z.shape[1:]), z.dtype)
        for z in ex["zero_outs"]
    ]
    out_arrs = ex["fn"](*concat_in, *concat_zeros)
    i = ex["out_names"].index("out")
    out = np.asarray(out_arrs[i]).reshape(n_cores, *ex["out_avals"][i].shape)
    return out


def kernel(**inputs) -> np.ndarray:
    return run(inputs)
